# revision 1
# baseline (speedup 1.0000x reference)
"""GAT layer (4 heads, mean-combined) on 8 Trainium2 NeuronCores.

Strategy (single SPMD program, all per-core variation lives in input data):
  - Edges are sharded by dst range: core m owns dst in [12500*m, 12500*(m+1)).
  - Reformulation: out[v] = mean_k (sum_e e_k*hw_k[src_e]) / (sum_e e_k),
    summed over edges with dst==v, where e_k = clip(exp(lrelu(logit)), ...).
    This removes the softmax's two-phase structure (no denominator gather).
  - Phase 1 (dense, replicated): haug = h @ [W_0..W_3 | W_k a1_k | W_k a2_k]
    -> DRAM table tabA[node] = [hw(256) | s_src(4) | pad] (1280B rows) and a
    per-core table tabS[local dst] = s_dst(4) (256B rows).
  - Phase 2 (edges): per 128-node window (98 per core), fetch per-edge rows
    with dma_gather (int16 idx, so tabA is addressed in 4 static 32768-row
    blocks; edges are grouped by (window, block) on the host and padded to
    128-multiples), compute e, weight messages in place, then segment-sum via
    an indicator matmul: PSUM[128 nodes, 260] += Ind_j.T @ Msg_j over the
    window's subtiles, where Ind = (rank_rel == iota). Flush PSUM -> SBUF,
    normalize by the accumulated denominators, DMA the 128 output rows.
No scatter primitives and no collectives are needed.
"""
import os

import numpy as np

import concourse.bass as bass
import concourse.bacc as bacc
import concourse.mybir as mybir
from concourse import tile
from concourse.bass import broadcast_tensor_aps
from concourse.bass_utils import run_bass_kernel_spmd

N = 100000
E = 1600000
IN_DIM = 128
OUT_DIM = 64
HEADS = 4
SLOPE = 0.2
CLIP_LO, CLIP_HI = 0.005, 10.0

NC = 8
NPC = N // NC            # 12500 dst nodes per core
WINR = 128               # ranks (local dst nodes) per window
NWIN = -(-NPC // WINR)   # 98 windows per core
NLOC = NWIN * WINR       # 12544 padded local nodes
SBLK = 32768             # src block size (int16 gather index limit)
NBLK = -(-N // SBLK)     # 4
NPAD = 49 * 2048         # 100352: padded node count for phase 1 tiling
ROW = 320                # legacy f32 row layout (debug emulation only)
BROW = 384               # tabB row (bf16): 256 hw + 8 (4 f32 s_src as bytes) + pad
USED = 260
SROW = 64                # tabS row (f32): 4 used
F32 = mybir.dt.float32
BF16 = mybir.dt.bfloat16
I16 = mybir.dt.int16
ADD, MUL, MIN, MAX, EQ = (
    mybir.AluOpType.add,
    mybir.AluOpType.mult,
    mybir.AluOpType.min,
    mybir.AluOpType.max,
    mybir.AluOpType.is_equal,
)


def _split_waits(nc):
    """This walrus build rejects >1 sync-wait per instruction. Hoist extras
    onto same-engine nops inserted immediately before the owner."""
    n = 0
    for f in nc.m.functions:
        for bb in f.blocks:
            new_list = []
            for ins in bb.instructions:
                si = ins.sync_info
                if si is not None and si.on_wait and len(si.on_wait) > 1:
                    waits = list(si.on_wait)
                    si.on_wait = waits[-1:]
                    for w in waits[:-1]:
                        nop = mybir.InstNoOp(
                            name=nc.get_next_instruction_name(),
                            engine=ins.engine,
                            sync_info=mybir.SyncInfo(on_wait=[w], on_update=[]),
                            bass_nofuse=True,
                        )
                        nc.register_instruction(nop)
                        new_list.append(nop)
                        n += 1
                new_list.append(ins)
            bb.instructions[:] = new_list
    return n


def _preprocess(h, edges, W, a):
    h = np.asarray(h, np.float32)
    W = np.asarray(W, np.float32)
    a = np.asarray(a, np.float32)
    src = np.asarray(edges[0], np.int64)
    dst = np.asarray(edges[1], np.int64)

    Ws = np.zeros((IN_DIM, 264), np.float32)
    for k in range(HEADS):
        Ws[:, OUT_DIM * k : OUT_DIM * (k + 1)] = W[k]
        Ws[:, 256 + k] = W[k] @ a[k, :OUT_DIM]
        Ws[:, 260 + k] = W[k] @ a[k, OUT_DIM:]

    hpad = np.zeros((NPAD, IN_DIM), np.float32)
    hpad[:N] = h
    hT = np.ascontiguousarray(hpad.T)  # [128, NPAD]

    # per-core own-range h, transposed, padded to NLOC
    hTo = np.zeros((NC, IN_DIM, NLOC), np.float32)
    for m in range(NC):
        lo = m * NPC
        hi = min(lo + NLOC, N)
        hTo[m, :, : hi - lo] = h[lo:hi].T

    m = dst // NPC
    dl = dst - m * NPC
    w = dl // WINR
    rr = dl - w * WINR
    blk = src >> 15
    lsrc = src - (blk << 15)

    # subtile capacity per block region (same for every window/core)
    key = (m * NWIN + w) * NBLK + blk
    cnt = np.bincount(key, minlength=NC * NWIN * NBLK).reshape(NC, NWIN, NBLK)
    wsubs = np.maximum(-(-cnt.max(axis=(0, 1)) // 128), 1)  # [NBLK]
    offs = np.concatenate(([0], np.cumsum(wsubs)))          # subtile offsets
    WSUB = int(offs[-1])
    SLOTS = WINR * WSUB                                     # slots per window

    # slot id for each edge: within (m, w, b) groups placed at region offsets
    order = np.lexsort((blk, w, m))
    so, wo, mo = src[order], w[order], m[order]
    rro, blko, lsrco = rr[order], blk[order], lsrc[order]
    keyo = (mo * NWIN + wo) * NBLK + blko
    starts = np.zeros(NC * NWIN * NBLK, np.int64)
    flat_cnt = cnt.reshape(-1)
    starts[1:] = np.cumsum(flat_cnt)[:-1]
    within = np.arange(E) - starts[keyo]
    slot = wo * SLOTS + offs[blko] * 128 + within           # within-core slot

    gidx = np.zeros((NC, NWIN * SLOTS), np.int16)
    rrf = np.full((NC, NWIN * SLOTS), -1.0, np.float32)
    gidx[mo, slot] = lsrco.astype(np.int16)
    rrf[mo, slot] = rro.astype(np.float32)

    # wrap16 packing for int16 idx streams ([16, n/16] tiled to 128 partitions)
    def wrap16(x):
        t = x.reshape(NC, -1, 16).transpose(0, 2, 1)
        return np.ascontiguousarray(np.tile(t, (1, 8, 1)))

    gidx_p = wrap16(gidx)                                   # [NC, 128, NWIN*SLOTS/16]
    # rank_rel per (partition=slot%128, col=w*WSUB + slot//128)
    rr_p = np.ascontiguousarray(
        rrf.reshape(NC, NWIN * WSUB, 128).transpose(0, 2, 1)
    )                                                       # [NC, 128, NWIN*WSUB]

    iota = np.tile(np.arange(128, dtype=np.float32), (128, 1))
    ident = np.eye(128, dtype=np.float32)

    shared = {"hT": hT, "Ws": Ws, "iota": iota, "ident": ident}
    per_core = [
        {
            "hTo": np.ascontiguousarray(hTo[m_]),
            "gidx": gidx_p[m_],
            "rr": rr_p[m_],
        }
        for m_ in range(NC)
    ]
    return shared, per_core, [int(x) for x in wsubs]


def _build(wsubs):
    WSUB = sum(wsubs)
    offs = [0]
    for x in wsubs:
        offs.append(offs[-1] + x)
    SLOTS = WINR * WSUB

    nc = bacc.Bacc(None)
    hT_d = nc.declare_dram_parameter("hT", [IN_DIM, NPAD], F32, isOutput=False)
    hTo_d = nc.declare_dram_parameter("hTo", [IN_DIM, NLOC], F32, isOutput=False)
    Ws_d = nc.declare_dram_parameter("Ws", [IN_DIM, 264], F32, isOutput=False)
    iota_d = nc.declare_dram_parameter("iota", [128, 128], F32, isOutput=False)
    ident_d = nc.declare_dram_parameter("ident", [128, 128], F32, isOutput=False)
    gidx_d = nc.declare_dram_parameter(
        "gidx", [128, NWIN * SLOTS // 16], I16, isOutput=False
    )
    rr_d = nc.declare_dram_parameter("rr", [128, NWIN * WSUB], F32, isOutput=False)
    out_d = nc.declare_dram_parameter("out", [NLOC, OUT_DIM], F32, isOutput=True)

    dbg = bool(os.environ.get("GAT_DEBUG"))
    if dbg:
        dbgA = nc.declare_dram_parameter("dbgA", [256, ROW], F32, isOutput=True)
        dbgS = nc.declare_dram_parameter("dbgS", [128, SROW], F32, isOutput=True)
        dbgG = nc.declare_dram_parameter(
            "dbgG", [128, sum(wsubs) * ROW], F32, isOutput=True
        )
        dbgE = nc.declare_dram_parameter(
            "dbgE", [128, sum(wsubs) * 4], F32, isOutput=True
        )
        dbgF = nc.declare_dram_parameter("dbgF", [128, USED], F32, isOutput=True)

    tabA = nc.dram_tensor("tabA", [NPAD, ROW], F32)
    tabS = nc.dram_tensor("tabS", [NLOC, SROW], F32)

    with tile.TileContext(nc) as tc:
        with tc.tile_pool(name="const", bufs=1) as cpool:
            Ws_t = cpool.tile_from(Ws_d[:])
            iota_t = cpool.tile_from(iota_d[:])
            ident_t = cpool.tile_from(ident_d[:])
            s2st = cpool.tile([128, NWIN * 4], F32)

            # ---- phase 1a: haug = h @ Ws -> tabA ----
            with (
                tc.tile_pool(name="p1h", bufs=2) as p1h,
                tc.tile_pool(name="p1ps", bufs=4, space="PSUM") as p1ps,
                tc.tile_pool(name="p1st", bufs=4) as p1st,
            ):
                for c in range(NPAD // 2048):
                    hc = p1h.tile([128, 2048], F32, tag="hc")
                    nc.sync.dma_start(hc[:], hT_d[:, c * 2048 : (c + 1) * 2048])
                    for i in range(16):
                        t = c * 16 + i
                        ps = p1ps.tile([128, USED], F32, tag="ps")
                        nc.tensor.matmul(
                            ps[:],
                            hc[:, i * 128 : (i + 1) * 128],
                            Ws_t[:, 0:USED],
                            start=True,
                            stop=True,
                        )
                        st = p1st.tile([128, USED], F32, tag="st")
                        nc.vector.tensor_copy(st[:], ps[:])
                        nc.sync.dma_start(
                            tabA[t * 128 : (t + 1) * 128, 0:USED], st[:]
                        )

                # ---- phase 1b: s_dst for own dst range -> tabS ----
                hto = p1h.tile([128, NLOC], F32, tag="hto")
                nc.sync.dma_start(hto[:], hTo_d[:])
                for t2 in range(NWIN):
                    ps2 = p1ps.tile([128, 4], F32, tag="ps2")
                    nc.tensor.matmul(
                        ps2[:],
                        hto[:, t2 * 128 : (t2 + 1) * 128],
                        Ws_t[:, 260:264],
                        start=True,
                        stop=True,
                    )
                    nc.vector.tensor_copy(s2st[:, t2 * 4 : (t2 + 1) * 4], ps2[:])
                    nc.sync.dma_start(
                        tabS[t2 * 128 : (t2 + 1) * 128, 0:4],
                        s2st[:, t2 * 4 : (t2 + 1) * 4],
                    )

            # ---- phase 2: per-window edge processing ----
            with (
                tc.tile_pool(name="pG", bufs=2) as pG,
                tc.tile_pool(name="pD", bufs=2) as pD,
                tc.tile_pool(name="pI", bufs=2) as pI,
                tc.tile_pool(name="pidx", bufs=2) as pidx,
                tc.tile_pool(name="pe", bufs=2) as pe,
                tc.tile_pool(name="pps", bufs=2, space="PSUM") as pps,
                tc.tile_pool(name="ppsT", bufs=1, space="PSUM") as ppsT,
                tc.tile_pool(name="ppsE", bufs=1, space="PSUM") as ppsE,
                tc.tile_pool(name="pIT", bufs=2) as pIT,
                tc.tile_pool(name="pf", bufs=2) as pf,
            ):
                GCOLS = SLOTS // 16
                for w in range(NWIN):
                    gi = pidx.tile([128, GCOLS], I16, tag="gi")
                    nc.sync.dma_start(
                        gi[:], gidx_d[:, w * GCOLS : (w + 1) * GCOLS]
                    )
                    rrt = pidx.tile([128, WSUB], F32, tag="rrt")
                    nc.sync.dma_start(rrt[:], rr_d[:, w * WSUB : (w + 1) * WSUB])

                    # indicator (built early: also used to expand s_dst)
                    Ind = pI.tile([128, WSUB * 128], F32, tag="Ind")
                    Ind3 = Ind[:].rearrange("p (s c) -> p s c", c=128)
                    rb, ib = broadcast_tensor_aps(
                        rrt[:].rearrange("p (s o) -> p s o", o=1),
                        iota_t[:].rearrange("(p o) c -> p o c", o=1),
                    )
                    nc.vector.tensor_tensor(Ind3, rb, ib, EQ)

                    # s_dst expansion: Dex = Ind @ s_win via PE transposes
                    swin = pD.tile([128, 4], F32, tag="swin")
                    nc.sync.dma_start(swin[:], tabS[w * 128 : (w + 1) * 128, 0:4])
                    psT = ppsT.tile([128, WSUB * 128], F32, tag="psT")
                    psT3 = psT[:].rearrange("p (s c) -> p s c", c=128)
                    for j in range(WSUB):
                        nc.tensor.transpose(psT3[:, j, :], Ind3[:, j, :], ident_t[:])
                    IndT = pIT.tile([128, WSUB * 128], F32, tag="IndT")
                    nc.vector.tensor_copy(IndT[:], psT[:])
                    IndT3 = IndT[:].rearrange("p (s c) -> p s c", c=128)
                    psE = ppsE.tile([128, WSUB * 4], F32, tag="psE")
                    for j in range(WSUB):
                        nc.tensor.matmul(
                            psE[:, 4 * j : 4 * j + 4],
                            IndT3[:, j, :],
                            swin[:],
                            start=True,
                            stop=True,
                        )
                    Dex = pD.tile([128, WSUB * 4], F32, tag="Dex")
                    nc.vector.tensor_copy(Dex[:], psE[:])
                    D3 = Dex[:].rearrange("p (s f) -> p s f", f=4)

                    G = pG.tile([128, WSUB * ROW], F32, tag="G")
                    G3 = G[:].rearrange("p (s e) -> p s e", e=ROW)
                    for b in range(NBLK):
                        nidx = 128 * wsubs[b]
                        lo = SBLK * b
                        hi = min(SBLK * (b + 1), NPAD)
                        nc.gpsimd.dma_gather(
                            G3[:, offs[b] : offs[b + 1], :],
                            tabA[lo:hi, :],
                            gi[:, offs[b] * 8 : offs[b + 1] * 8],
                            nidx,
                            nidx,
                            ROW,
                        )

                    # e = clip(exp(lrelu(s_src + s_dst)))
                    L = pe.tile([128, WSUB * 4], F32, tag="L")
                    L3 = L[:].rearrange("p (s f) -> p s f", f=4)
                    nc.vector.tensor_tensor(
                        L3, G3[:, :, 256:260], D3[:, :, 0:4], ADD
                    )
                    L2 = pe.tile([128, WSUB * 4], F32, tag="L2")
                    nc.vector.scalar_tensor_tensor(
                        L2[:], L[:], SLOPE, L[:], MUL, MAX
                    )
                    Et = pe.tile([128, WSUB * 4], F32, tag="Et")
                    nc.scalar.activation(
                        Et[:], L2[:], mybir.ActivationFunctionType.Exp
                    )
                    nc.vector.tensor_scalar(Et[:], Et[:], CLIP_HI, CLIP_LO, MIN, MAX)
                    E3 = Et[:].rearrange("p (s f) -> p s f", f=4)

                    # weight messages in place; append e to columns 256:260
                    for k in range(HEADS):
                        gk = G3[:, :, OUT_DIM * k : OUT_DIM * (k + 1)]
                        ek, gkb = broadcast_tensor_aps(E3[:, :, k : k + 1], gk)
                        nc.vector.tensor_tensor(gkb, gkb, ek, MUL)
                    nc.vector.tensor_copy(G3[:, :, 256:260], E3)

                    # segment matmuls
                    ps = pps.tile([128, USED], F32, tag="mps")
                    for j in range(WSUB):
                        nc.tensor.matmul(
                            ps[:],
                            Ind3[:, j, :],
                            G3[:, j, 0:USED],
                            start=(j == 0),
                            stop=(j == WSUB - 1),
                        )

                    # flush + normalize: out = 0.25 * sum_k num_k / den_k
                    F = pf.tile([128, USED], F32, tag="F")
                    nc.vector.tensor_copy(F[:], ps[:])
                    R = pf.tile([128, 4], F32, tag="R")
                    nc.vector.tensor_scalar(R[:], F[:, 256:260], 1e-30, None, MAX)
                    R2 = pf.tile([128, 4], F32, tag="R2")
                    nc.vector.reciprocal(R2[:], R[:])
                    nc.vector.tensor_scalar(R2[:], R2[:], 1.0 / HEADS, None, MUL)
                    O = pf.tile([128, OUT_DIM], F32, tag="O")
                    nc.vector.tensor_scalar(
                        O[:], F[:, 0:OUT_DIM], R2[:, 0:1], None, MUL
                    )
                    for k in range(1, HEADS):
                        nc.vector.scalar_tensor_tensor(
                            O[:],
                            F[:, OUT_DIM * k : OUT_DIM * (k + 1)],
                            R2[:, k : k + 1],
                            O[:],
                            MUL,
                            ADD,
                        )
                    nc.sync.dma_start(out_d[w * 128 : (w + 1) * 128, :], O[:])
                    if dbg and w == 0:
                        nc.sync.dma_start(dbgG[:], G[:])
                        nc.sync.dma_start(dbgE[:], Et[:])
                        nc.sync.dma_start(dbgF[:], F[:])
                if dbg:
                    nc.sync.dma_start(dbgA[:], tabA[0:256, :])
                    nc.sync.dma_start(dbgS[:], tabS[0:128, :])

    nc.compile()
    _split_waits(nc)
    return nc


def kernel(h, edges, W, a):
    shared, per_core, wsubs = _preprocess(h, edges, W, a)
    nc = _build(wsubs)
    in_maps = [{**shared, **pc} for pc in per_core]
    r = run_bass_kernel_spmd(
        nc, in_maps, list(range(NC)), trace=bool(os.environ.get("GAT_TRACE"))
    )
    res = r.results
    global _last_results, _last_exec_ns, _last_bkr
    _last_results = res
    _last_exec_ns = r.exec_time_ns
    _last_bkr = r
    out = np.zeros((N, OUT_DIM), np.float32)
    for m in range(NC):
        out[m * NPC : (m + 1) * NPC] = res[m]["out"][:NPC]
    return out



# revision 3
# speedup vs baseline: 1.0090x; 1.0090x over previous
"""GAT layer (4 heads, mean-combined) on 8 Trainium2 NeuronCores.

v2 strategy (single SPMD program; per-core variation lives in input data):
  - Edges sharded by dst range: core m owns dst in [12500*m, 12500*(m+1)).
  - out[v] = mean_k (sum_e e_k*hw_k[src_e]) / (sum_e e_k) over edges with
    dst==v, e_k = clip(exp(lrelu(s_src+s_dst)), lo, hi). No softmax gather.
  - Phase 1 (bf16): haug = h @ [W_0..W_3 | W_k a1_k] -> DRAM table
    tabA[node] = [hw(256 bf16) | s_src(4 bf16) | pad] (768B rows).
    s_dst for the core's own dst range stays resident in SBUF (s2st).
  - Phase 2: per 128-dst window, fetch per-edge rows with dma_gather
    (bf16, int16 idx over 4 static 32768-row blocks; edges grouped by
    (window, block), sorted by src id, padded to 128-multiples with idx=-1
    so pad rows are skipped). Compute e, weight messages in place, then
    segment-sum via an indicator matmul PSUM[rank,260] += Ind_j.T @ Msg_j.
    Both Ind (slot-major) and IndT (rank-major, for the s_dst expansion
    psE = IndT_j.T @ swin) are built directly on DVE via IS_EQ against
    iota constants; IndT uses a host-replicated rank stream (rrT).
  - Subtile counts per (window, block) are the max over cores (static
    program), variable across windows to cut padding.
No scatter primitives and no collectives are needed.
"""
import os

import numpy as np
import ml_dtypes

import concourse.bass as bass
import concourse.bacc as bacc
import concourse.mybir as mybir
from concourse import tile
from concourse.bass import broadcast_tensor_aps
from concourse.bass_utils import run_bass_kernel_spmd

N = 100000
E = 1600000
IN_DIM = 128
OUT_DIM = 64
HEADS = 4
SLOPE = 0.2
CLIP_LO, CLIP_HI = 0.005, 10.0

NC = 8
NPC = N // NC            # 12500 dst nodes per core
WINR = 128               # ranks (local dst nodes) per window
NWIN = -(-NPC // WINR)   # 98 windows per core
NLOC = NWIN * WINR       # 12544 padded local nodes
SBLK = 32768             # src block size (int16 gather index limit)
NBLK = -(-N // SBLK)     # 4
NPAD = 49 * 2048         # 100352: padded node count for phase 1 tiling
BROW = 384               # tabA row in bf16 elems: 256 hw | 4 s_src | pad
USED = 260
BF = ml_dtypes.bfloat16
F32 = mybir.dt.float32
BF16 = mybir.dt.bfloat16
I16 = mybir.dt.int16
ADD, MUL, MIN, MAX, EQ = (
    mybir.AluOpType.add,
    mybir.AluOpType.mult,
    mybir.AluOpType.min,
    mybir.AluOpType.max,
    mybir.AluOpType.is_equal,
)


def _split_waits(nc):
    """This walrus build rejects >1 sync-wait per instruction. Hoist extras
    onto same-engine nops inserted immediately before the owner."""
    n = 0
    for f in nc.m.functions:
        for bb in f.blocks:
            new_list = []
            for ins in bb.instructions:
                si = ins.sync_info
                if si is not None and si.on_wait and len(si.on_wait) > 1:
                    waits = list(si.on_wait)
                    si.on_wait = waits[-1:]
                    for w in waits[:-1]:
                        nop = mybir.InstNoOp(
                            name=nc.get_next_instruction_name(),
                            engine=ins.engine,
                            sync_info=mybir.SyncInfo(on_wait=[w], on_update=[]),
                            bass_nofuse=True,
                        )
                        nc.register_instruction(nop)
                        new_list.append(nop)
                        n += 1
                new_list.append(ins)
            bb.instructions[:] = new_list
    return n


def _offsets(WS):
    """Per-window block offsets / totals from the [NWIN, NBLK] subtile grid."""
    boff = []
    woff = [0]
    for w in range(len(WS)):
        acc = [0]
        for b in range(NBLK):
            acc.append(acc[-1] + WS[w][b])
        boff.append(acc)
        woff.append(woff[-1] + acc[-1])
    return boff, woff


def _preprocess(h, edges, W, a):
    h = np.asarray(h, np.float32)
    W = np.asarray(W, np.float32)
    a = np.asarray(a, np.float32)
    src = np.asarray(edges[0], np.int64)
    dst = np.asarray(edges[1], np.int64)

    Ws = np.zeros((IN_DIM, 264), np.float32)
    for k in range(HEADS):
        Ws[:, OUT_DIM * k : OUT_DIM * (k + 1)] = W[k]
        Ws[:, 256 + k] = W[k] @ a[k, :OUT_DIM]
        Ws[:, 260 + k] = W[k] @ a[k, OUT_DIM:]

    hpad = np.zeros((NPAD, IN_DIM), np.float32)
    hpad[:N] = h
    hT = np.ascontiguousarray(hpad.T.astype(BF))  # [128, NPAD] bf16

    # per-core own-range h, transposed, padded to NLOC
    hTo = np.zeros((NC, IN_DIM, NLOC), BF)
    for m in range(NC):
        lo = m * NPC
        hi = min(lo + NLOC, N)
        hTo[m, :, : hi - lo] = h[lo:hi].T.astype(BF)

    m = dst // NPC
    dl = dst - m * NPC
    w = dl // WINR
    rr = dl - w * WINR
    blk = src >> 15
    lsrc = src - (blk << 15)

    # static subtile capacity per (window, block): max over cores
    key = (m * NWIN + w) * NBLK + blk
    cnt = np.bincount(key, minlength=NC * NWIN * NBLK).reshape(NC, NWIN, NBLK)
    WS = (-(-cnt.max(axis=0) // 128)).astype(np.int64)       # [NWIN, NBLK]
    boff, woff = _offsets(WS.tolist())
    TOTSUB = woff[-1]

    # slot id for each edge: within (m, w, b) groups, sorted by src id
    order = np.lexsort((lsrc, blk, w, m))
    wo, mo = w[order], m[order]
    rro, blko, lsrco = rr[order], blk[order], lsrc[order]
    keyo = (mo * NWIN + wo) * NBLK + blko
    starts = np.zeros(NC * NWIN * NBLK, np.int64)
    flat_cnt = cnt.reshape(-1)
    starts[1:] = np.cumsum(flat_cnt)[:-1]
    within = np.arange(E) - starts[keyo]
    woff_a = np.asarray(woff[:-1])
    boff_a = np.asarray([row[:-1] for row in boff])          # [NWIN, NBLK]
    regbase = (woff_a[wo] + boff_a[wo, blko]) * 128
    slot = regbase + within                                  # within-core slot

    gidx = np.zeros((NC, TOTSUB * 128), np.int16)
    rrf = np.full((NC, TOTSUB * 128), -1.0, np.float32)
    gidx[mo, slot] = lsrco.astype(np.int16)
    rrf[mo, slot] = rro.astype(np.float32)

    # wrap16 packing for int16 idx streams ([16, n/16] tiled to 128 parts)
    t = gidx.reshape(NC, -1, 16).transpose(0, 2, 1)
    gidx_p = np.ascontiguousarray(np.tile(t, (1, 8, 1)))     # [NC,128,TOTSUB*8]
    # rank per slot, slot-major: rr_p[m, p, j] = rank of slot j*128+p
    rr_p = np.ascontiguousarray(
        rrf.reshape(NC, TOTSUB, 128).transpose(0, 2, 1).astype(BF)
    )                                                        # [NC, 128, TOTSUB]
    # rank stream replicated across partitions (for direct IndT build)
    rrT_p = np.ascontiguousarray(
        np.broadcast_to(rrf.astype(BF)[:, None, :], (NC, 128, TOTSUB * 128))
    )                                                        # [NC,128,TOTSUB*128]

    iota = np.tile(np.arange(128, dtype=BF), (128, 1))
    iotaP = np.ascontiguousarray(np.arange(128, dtype=BF).reshape(128, 1))

    shared = {
        "hT": hT,
        "Ws": np.ascontiguousarray(Ws.astype(BF)),
        "iota": np.ascontiguousarray(iota),
        "iotaP": iotaP,
    }
    per_core = [
        {
            "hTo": np.ascontiguousarray(hTo[m_]),
            "gidx": gidx_p[m_],
            "rr": rr_p[m_],
            "rrT": rrT_p[m_],
        }
        for m_ in range(NC)
    ]
    return shared, per_core, WS.tolist()


def _build(WS):
    boff, woff = _offsets(WS)
    TOTSUB = woff[-1]
    WSUBMAX = max(woff[i + 1] - woff[i] for i in range(NWIN))

    nc = bacc.Bacc(None)
    hT_d = nc.declare_dram_parameter("hT", [IN_DIM, NPAD], BF16, isOutput=False)
    hTo_d = nc.declare_dram_parameter("hTo", [IN_DIM, NLOC], BF16, isOutput=False)
    Ws_d = nc.declare_dram_parameter("Ws", [IN_DIM, 264], BF16, isOutput=False)
    iota_d = nc.declare_dram_parameter("iota", [128, 128], BF16, isOutput=False)
    iotaP_d = nc.declare_dram_parameter("iotaP", [128, 1], BF16, isOutput=False)
    gidx_d = nc.declare_dram_parameter(
        "gidx", [128, TOTSUB * 8], I16, isOutput=False
    )
    rr_d = nc.declare_dram_parameter("rr", [128, TOTSUB], BF16, isOutput=False)
    rrT_d = nc.declare_dram_parameter(
        "rrT", [128, TOTSUB * 128], BF16, isOutput=False
    )
    out_d = nc.declare_dram_parameter("out", [NLOC, OUT_DIM], F32, isOutput=True)

    tabA = nc.dram_tensor("tabA", [NPAD, BROW], BF16)

    with tile.TileContext(nc) as tc:
        with tc.tile_pool(name="const", bufs=1) as cpool:
            Ws_t = cpool.tile_from(Ws_d[:])
            iota_t = cpool.tile_from(iota_d[:])
            iotaP_t = cpool.tile_from(iotaP_d[:])
            s2st = cpool.tile([128, NWIN * 4], BF16)
            gi_all = cpool.tile([128, TOTSUB * 8], I16)
            rr_all = cpool.tile([128, TOTSUB], BF16)
            nc.sync.dma_start(gi_all[:], gidx_d[:])
            nc.sync.dma_start(rr_all[:], rr_d[:])

            # ---- phase 1a: haug = h @ Ws -> tabA (bf16 rows) ----
            with (
                tc.tile_pool(name="p1h", bufs=2) as p1h,
                tc.tile_pool(name="p1ps", bufs=4, space="PSUM") as p1ps,
                tc.tile_pool(name="p1st", bufs=4) as p1st,
            ):
                for c in range(NPAD // 2048):
                    hc = p1h.tile([128, 2048], BF16, tag="hc")
                    nc.sync.dma_start(hc[:], hT_d[:, c * 2048 : (c + 1) * 2048])
                    for i in range(16):
                        t = c * 16 + i
                        ps = p1ps.tile([128, USED], F32, tag="ps")
                        nc.tensor.matmul(
                            ps[:],
                            hc[:, i * 128 : (i + 1) * 128],
                            Ws_t[:, 0:USED],
                            start=True,
                            stop=True,
                        )
                        st = p1st.tile([128, USED], BF16, tag="st")
                        nc.vector.tensor_copy(st[:], ps[:])
                        nc.sync.dma_start(
                            tabA[t * 128 : (t + 1) * 128, 0:USED], st[:]
                        )

                # ---- phase 1b: s_dst for own dst range -> SBUF (s2st) ----
                hto = p1h.tile([128, NLOC], BF16, tag="hto")
                nc.sync.dma_start(hto[:], hTo_d[:])
                for t2 in range(NWIN):
                    ps2 = p1ps.tile([128, 4], F32, tag="ps2")
                    nc.tensor.matmul(
                        ps2[:],
                        hto[:, t2 * 128 : (t2 + 1) * 128],
                        Ws_t[:, 260:264],
                        start=True,
                        stop=True,
                    )
                    nc.vector.tensor_copy(s2st[:, t2 * 4 : (t2 + 1) * 4], ps2[:])

            # ---- phase 2: per-window edge processing ----
            with (
                tc.tile_pool(name="pG", bufs=2) as pG,
                tc.tile_pool(name="prT", bufs=2) as prT,
                tc.tile_pool(name="pI", bufs=2) as pI,
                tc.tile_pool(name="pIT", bufs=2) as pIT,
                tc.tile_pool(name="pD", bufs=2) as pD,
                tc.tile_pool(name="pe", bufs=2) as pe,
                tc.tile_pool(name="pps", bufs=2, space="PSUM") as pps,
                tc.tile_pool(name="ppsE", bufs=2, space="PSUM") as ppsE,
                tc.tile_pool(name="pf", bufs=2) as pf,
            ):
                for w in range(NWIN):
                    WB = woff[w + 1] - woff[w]
                    j0 = woff[w]

                    # indicator Ind[slot, rank] and its transpose, via IS_EQ
                    Ind = pI.tile([128, WSUBMAX * 128], BF16, tag="Ind")
                    Ind3 = Ind[:, 0 : WB * 128].rearrange(
                        "p (s c) -> p s c", c=128
                    )
                    rb, ib = broadcast_tensor_aps(
                        rr_all[:, j0 : j0 + WB].rearrange(
                            "p (s o) -> p s o", o=1
                        ),
                        iota_t[:].rearrange("(p o) c -> p o c", o=1),
                    )
                    nc.vector.tensor_tensor(Ind3, rb, ib, EQ)

                    rrTb = prT.tile([128, WSUBMAX * 128], BF16, tag="rrTb")
                    nc.sync.dma_start(
                        rrTb[:, 0 : WB * 128],
                        rrT_d[:, j0 * 128 : (j0 + WB) * 128],
                    )
                    IndT = pIT.tile([128, WSUBMAX * 128], BF16, tag="IndT")
                    tb, pb = broadcast_tensor_aps(
                        rrTb[:, 0 : WB * 128], iotaP_t[:]
                    )
                    nc.vector.tensor_tensor(IndT[:, 0 : WB * 128], tb, pb, EQ)
                    IndT3 = IndT[:, 0 : WB * 128].rearrange(
                        "p (s c) -> p s c", c=128
                    )

                    # s_dst expansion: Dex[slot, k] = s_dst[rank(slot), k]
                    psE = ppsE.tile([128, WSUBMAX * 4], F32, tag="psE")
                    for j in range(WB):
                        nc.tensor.matmul(
                            psE[:, 4 * j : 4 * j + 4],
                            IndT3[:, j, :],
                            s2st[:, 4 * w : 4 * w + 4],
                            start=True,
                            stop=True,
                        )
                    Dex = pD.tile([128, WSUBMAX * 4], BF16, tag="Dex")
                    nc.vector.tensor_copy(Dex[:, 0 : WB * 4], psE[:, 0 : WB * 4])
                    D3 = Dex[:, 0 : WB * 4].rearrange("p (s f) -> p s f", f=4)

                    # gather per-edge rows (pad slots have idx=-1: skipped)
                    G = pG.tile([128, WSUBMAX * BROW], BF16, tag="G")
                    if w < 2:
                        nc.vector.memset(G[:], 0.0)
                    G3 = G[:, 0 : WB * BROW].rearrange(
                        "p (s e) -> p s e", e=BROW
                    )
                    for b in range(NBLK):
                        ws = WS[w][b]
                        if ws == 0:
                            continue
                        gj0 = j0 + boff[w][b]
                        nidx = 128 * ws
                        lo = SBLK * b
                        hi = min(SBLK * (b + 1), NPAD)
                        nc.gpsimd.dma_gather(
                            G3[:, boff[w][b] : boff[w][b] + ws, :],
                            tabA[lo:hi, :],
                            gi_all[:, gj0 * 8 : (gj0 + ws) * 8],
                            nidx,
                            nidx,
                            BROW,
                        )

                    # e = clip(exp(lrelu(s_src + s_dst)))
                    L = pe.tile([128, WSUBMAX * 4], F32, tag="L")
                    L3 = L[:, 0 : WB * 4].rearrange("p (s f) -> p s f", f=4)
                    nc.vector.tensor_tensor(
                        L3, G3[:, :, 256:260], D3[:, :, 0:4], ADD
                    )
                    L2 = pe.tile([128, WSUBMAX * 4], F32, tag="L2")
                    nc.vector.scalar_tensor_tensor(
                        L2[:, 0 : WB * 4],
                        L[:, 0 : WB * 4],
                        SLOPE,
                        L[:, 0 : WB * 4],
                        MUL,
                        MAX,
                    )
                    Et = pe.tile([128, WSUBMAX * 4], F32, tag="Et")
                    nc.scalar.activation(
                        Et[:, 0 : WB * 4],
                        L2[:, 0 : WB * 4],
                        mybir.ActivationFunctionType.Exp,
                    )
                    Ec = pe.tile([128, WSUBMAX * 4], F32, tag="Ec")
                    nc.vector.tensor_scalar(
                        Ec[:, 0 : WB * 4], Et[:, 0 : WB * 4], CLIP_HI, None, MIN
                    )
                    Eb = pe.tile([128, WSUBMAX * 4], BF16, tag="Eb")
                    nc.vector.tensor_scalar(
                        Eb[:, 0 : WB * 4], Ec[:, 0 : WB * 4], CLIP_LO, None, MAX
                    )
                    E3 = Eb[:, 0 : WB * 4].rearrange("p (s f) -> p s f", f=4)

                    # weight messages in place; append e to columns 256:260
                    for k in range(HEADS):
                        gk = G3[:, :, OUT_DIM * k : OUT_DIM * (k + 1)]
                        ek, gkb = broadcast_tensor_aps(E3[:, :, k : k + 1], gk)
                        nc.vector.tensor_tensor(gkb, gkb, ek, MUL)
                    nc.vector.tensor_copy(G3[:, :, 256:260], E3)

                    # segment matmuls
                    ps = pps.tile([128, USED], F32, tag="mps")
                    for j in range(WB):
                        nc.tensor.matmul(
                            ps[:],
                            Ind3[:, j, :],
                            G3[:, j, 0:USED],
                            start=(j == 0),
                            stop=(j == WB - 1),
                        )

                    # flush + normalize: out = 0.25 * sum_k num_k / den_k
                    F = pf.tile([128, USED], F32, tag="F")
                    nc.vector.tensor_copy(F[:], ps[:])
                    R = pf.tile([128, 4], F32, tag="R")
                    nc.vector.tensor_scalar(R[:], F[:, 256:260], 1e-30, None, MAX)
                    R2 = pf.tile([128, 4], F32, tag="R2")
                    nc.vector.reciprocal(R2[:], R[:])
                    nc.vector.tensor_scalar(R2[:], R2[:], 1.0 / HEADS, None, MUL)
                    O = pf.tile([128, OUT_DIM], F32, tag="O")
                    nc.vector.tensor_scalar(
                        O[:], F[:, 0:OUT_DIM], R2[:, 0:1], None, MUL
                    )
                    for k in range(1, HEADS):
                        nc.vector.scalar_tensor_tensor(
                            O[:],
                            F[:, OUT_DIM * k : OUT_DIM * (k + 1)],
                            R2[:, k : k + 1],
                            O[:],
                            MUL,
                            ADD,
                        )
                    nc.sync.dma_start(out_d[w * 128 : (w + 1) * 128, :], O[:])

    nc.compile()
    _split_waits(nc)
    return nc


def kernel(h, edges, W, a):
    shared, per_core, WS = _preprocess(h, edges, W, a)
    nc = _build(WS)
    in_maps = [{**shared, **pc} for pc in per_core]
    r = run_bass_kernel_spmd(
        nc, in_maps, list(range(NC)), trace=bool(os.environ.get("GAT_TRACE"))
    )
    res = r.results
    global _last_results, _last_exec_ns, _last_bkr
    _last_results = res
    _last_exec_ns = r.exec_time_ns
    _last_bkr = r
    out = np.zeros((N, OUT_DIM), np.float32)
    for m in range(NC):
        out[m * NPC : (m + 1) * NPC] = res[m]["out"][:NPC]
    return out


# revision 13
# speedup vs baseline: 1.2572x; 1.2459x over previous
"""GAT layer (4 heads, mean-combined) on 8 Trainium2 NeuronCores.

v2 strategy (single SPMD program; per-core variation lives in input data):
  - Edges sharded by dst range: core m owns dst in [12500*m, 12500*(m+1)).
  - out[v] = mean_k (sum_e e_k*hw_k[src_e]) / (sum_e e_k) over edges with
    dst==v, e_k = clip(exp(lrelu(s_src+s_dst)), lo, hi). No softmax gather.
  - Phase 1 (bf16): haug = h @ [W_0..W_3 | W_k a1_k] -> DRAM table
    tabA[node] = [hw(256 bf16) | s_src(4 bf16) | pad] (768B rows).
    s_dst for the core's own dst range stays resident in SBUF (s2st).
  - Phase 2: per 128-dst window, fetch per-edge rows with dma_gather
    (bf16, int16 idx over 4 static 32768-row blocks; edges grouped by
    (window, block), sorted by src id, padded to 128-multiples with idx=-1
    so pad rows are skipped). Compute e, weight messages in place, then
    segment-sum via an indicator matmul PSUM[rank,260] += Ind_j.T @ Msg_j.
    Both Ind (slot-major) and IndT (rank-major, for the s_dst expansion
    psE = IndT_j.T @ swin) are built directly on DVE via IS_EQ against
    iota constants; IndT uses a host-replicated rank stream (rrT).
  - Subtile counts per (window, block) are the max over cores (static
    program), variable across windows to cut padding.
No scatter primitives and no collectives are needed.
"""
import os

import numpy as np
import ml_dtypes

import concourse.bass as bass
import concourse.bacc as bacc
import concourse.mybir as mybir
from concourse import tile
from concourse.bass import broadcast_tensor_aps
from concourse.bass_utils import run_bass_kernel_spmd

N = 100000
E = 1600000
IN_DIM = 128
OUT_DIM = 64
HEADS = 4
SLOPE = 0.2
CLIP_LO, CLIP_HI = 0.005, 10.0

NC = 8
NPC = N // NC            # 12500 dst nodes per core
WINR = 128               # ranks (local dst nodes) per window
NWIN = -(-NPC // WINR)   # 98 windows per core
NLOC = NWIN * WINR       # 12544 padded local nodes
SBLK = 32768             # src block size (int16 gather index limit)
NBLK = -(-N // SBLK)     # 4
NPAD = 49 * 2048         # 100352: padded node count for phase 1 tiling
BROW = 384               # tabA row in bf16 elems: 256 hw | 4 s_src | pad
USED = 260
BF = ml_dtypes.bfloat16
F32 = mybir.dt.float32
BF16 = mybir.dt.bfloat16
I16 = mybir.dt.int16
ADD, MUL, MIN, MAX, EQ = (
    mybir.AluOpType.add,
    mybir.AluOpType.mult,
    mybir.AluOpType.min,
    mybir.AluOpType.max,
    mybir.AluOpType.is_equal,
)


def _split_waits(nc):
    """This walrus build rejects >1 sync-wait per instruction. Hoist extras
    onto same-engine nops inserted immediately before the owner."""
    n = 0
    for f in nc.m.functions:
        for bb in f.blocks:
            new_list = []
            for ins in bb.instructions:
                si = ins.sync_info
                if si is not None and si.on_wait and len(si.on_wait) > 1:
                    waits = list(si.on_wait)
                    si.on_wait = waits[-1:]
                    for w in waits[:-1]:
                        nop = mybir.InstNoOp(
                            name=nc.get_next_instruction_name(),
                            engine=ins.engine,
                            sync_info=mybir.SyncInfo(on_wait=[w], on_update=[]),
                            bass_nofuse=True,
                        )
                        nc.register_instruction(nop)
                        new_list.append(nop)
                        n += 1
                new_list.append(ins)
            bb.instructions[:] = new_list
    return n


def _offsets(WS):
    """Per-window block offsets / totals from the [NWIN, NBLK] subtile grid."""
    boff = []
    woff = [0]
    for w in range(len(WS)):
        acc = [0]
        for b in range(NBLK):
            acc.append(acc[-1] + WS[w][b])
        boff.append(acc)
        woff.append(woff[-1] + acc[-1])
    return boff, woff


def _preprocess(h, edges, W, a):
    h = np.asarray(h, np.float32)
    W = np.asarray(W, np.float32)
    a = np.asarray(a, np.float32)
    src = np.asarray(edges[0], np.int64)
    dst = np.asarray(edges[1], np.int64)

    Ws = np.zeros((IN_DIM, 264), np.float32)
    for k in range(HEADS):
        Ws[:, OUT_DIM * k : OUT_DIM * (k + 1)] = W[k]
        Ws[:, 256 + k] = W[k] @ a[k, :OUT_DIM]
        Ws[:, 260 + k] = W[k] @ a[k, OUT_DIM:]
    ident = np.eye(128, dtype=BF)

    hpad = np.zeros((NPAD, IN_DIM), np.float32)
    hpad[:N] = h
    hT = np.ascontiguousarray(hpad.T.astype(BF))  # [128, NPAD] bf16

    # per-core own-range h, transposed, padded to NLOC
    hTo = np.zeros((NC, IN_DIM, NLOC), BF)
    for m in range(NC):
        lo = m * NPC
        hi = min(lo + NLOC, N)
        hTo[m, :, : hi - lo] = h[lo:hi].T.astype(BF)

    m = dst // NPC
    dl = dst - m * NPC
    w = dl // WINR
    rr = dl - w * WINR
    blk = src >> 15
    lsrc = src - (blk << 15)

    # static subtile capacity per (window, block): max over cores
    key = (m * NWIN + w) * NBLK + blk
    cnt = np.bincount(key, minlength=NC * NWIN * NBLK).reshape(NC, NWIN, NBLK)
    WS = (-(-cnt.max(axis=0) // 128)).astype(np.int64)       # [NWIN, NBLK]
    boff, woff = _offsets(WS.tolist())
    TOTSUB = woff[-1]

    # slot id for each edge: within (m, w, b) groups, sorted by src id
    order = np.lexsort((lsrc, blk, w, m))
    wo, mo = w[order], m[order]
    rro, blko, lsrco = rr[order], blk[order], lsrc[order]
    keyo = (mo * NWIN + wo) * NBLK + blko
    starts = np.zeros(NC * NWIN * NBLK, np.int64)
    flat_cnt = cnt.reshape(-1)
    starts[1:] = np.cumsum(flat_cnt)[:-1]
    within = np.arange(E) - starts[keyo]
    woff_a = np.asarray(woff[:-1])
    boff_a = np.asarray([row[:-1] for row in boff])          # [NWIN, NBLK]
    regbase = (woff_a[wo] + boff_a[wo, blko]) * 128
    slot = regbase + within                                  # within-core slot

    gidx = np.zeros((NC, TOTSUB * 128), np.int16)
    rrf = np.full((NC, TOTSUB * 128), -1.0, np.float32)
    gidx[mo, slot] = lsrco.astype(np.int16)
    rrf[mo, slot] = rro.astype(np.float32)

    # wrap16 packing for int16 idx streams ([16, n/16] tiled to 128 parts)
    t = gidx.reshape(NC, -1, 16).transpose(0, 2, 1)
    gidx_p = np.ascontiguousarray(np.tile(t, (1, 8, 1)))     # [NC,128,TOTSUB*8]
    # rank per slot, slot-major: rr_p[m, p, j] = rank of slot j*128+p
    rr_p = np.ascontiguousarray(
        rrf.reshape(NC, TOTSUB, 128).transpose(0, 2, 1).astype(BF)
    )                                                        # [NC, 128, TOTSUB]

    iota = np.tile(np.arange(128, dtype=BF), (128, 1))

    shared = {
        "hT": hT,
        "Ws": np.ascontiguousarray(Ws.astype(BF)),
        "iota": np.ascontiguousarray(iota),
        "ident": ident,
    }
    per_core = [
        {
            "hTo": np.ascontiguousarray(hTo[m_]),
            "gidx": gidx_p[m_],
            "rr": rr_p[m_],
        }
        for m_ in range(NC)
    ]
    return shared, per_core, WS.tolist()


def _build(WS):
    boff, woff = _offsets(WS)
    TOTSUB = woff[-1]
    WSUBMAX = max(woff[i + 1] - woff[i] for i in range(NWIN))

    nc = bacc.Bacc(None)
    hT_d = nc.declare_dram_parameter("hT", [IN_DIM, NPAD], BF16, isOutput=False)
    hTo_d = nc.declare_dram_parameter("hTo", [IN_DIM, NLOC], BF16, isOutput=False)
    Ws_d = nc.declare_dram_parameter("Ws", [IN_DIM, 264], BF16, isOutput=False)
    iota_d = nc.declare_dram_parameter("iota", [128, 128], BF16, isOutput=False)
    ident_d = nc.declare_dram_parameter("ident", [128, 128], BF16, isOutput=False)
    gidx_d = nc.declare_dram_parameter(
        "gidx", [128, TOTSUB * 8], I16, isOutput=False
    )
    rr_d = nc.declare_dram_parameter("rr", [128, TOTSUB], BF16, isOutput=False)
    out_d = nc.declare_dram_parameter("out", [NLOC, OUT_DIM], F32, isOutput=True)

    tabA = nc.dram_tensor("tabA", [NPAD, BROW], BF16)

    with tile.TileContext(nc) as tc:
        with tc.tile_pool(name="const", bufs=1) as cpool:
            Ws_t = cpool.tile_from(Ws_d[:])
            iota_t = cpool.tile_from(iota_d[:])
            ident_t = cpool.tile_from(ident_d[:])
            s2st = cpool.tile([128, NWIN * 4], BF16)
            gi_all = cpool.tile([128, TOTSUB * 8], I16)
            rr_all = cpool.tile([128, TOTSUB], BF16)
            nc.sync.dma_start(gi_all[:], gidx_d[:])
            nc.sync.dma_start(rr_all[:], rr_d[:])

            # ---- phase 1a: haug = h @ Ws -> tabA (bf16 rows) ----
            with (
                tc.tile_pool(name="p1h", bufs=2) as p1h,
                tc.tile_pool(name="p1ps", bufs=4, space="PSUM") as p1ps,
                tc.tile_pool(name="p1st", bufs=4) as p1st,
            ):
                for c in range(NPAD // 2048):
                    hc = p1h.tile([128, 2048], BF16, tag="hc")
                    nc.sync.dma_start(hc[:], hT_d[:, c * 2048 : (c + 1) * 2048])
                    for i in range(16):
                        t = c * 16 + i
                        ps = p1ps.tile([128, USED], F32, tag="ps")
                        nc.tensor.matmul(
                            ps[:],
                            hc[:, i * 128 : (i + 1) * 128],
                            Ws_t[:, 0:USED],
                            start=True,
                            stop=True,
                        )
                        st = p1st.tile([128, USED], BF16, tag="st")
                        if t % 2 == 0:
                            nc.vector.tensor_copy(st[:], ps[:])
                        else:
                            nc.scalar.activation(
                                st[:],
                                ps[:],
                                mybir.ActivationFunctionType.Copy,
                            )
                        nc.sync.dma_start(
                            tabA[t * 128 : (t + 1) * 128, 0:USED], st[:]
                        )

                # ---- phase 1b: s_dst for own dst range -> SBUF (s2st) ----
                hto = p1h.tile([128, NLOC], BF16, tag="hto")
                nc.sync.dma_start(hto[:], hTo_d[:])
                for t2 in range(NWIN):
                    ps2 = p1ps.tile([128, 4], F32, tag="ps2")
                    nc.tensor.matmul(
                        ps2[:],
                        hto[:, t2 * 128 : (t2 + 1) * 128],
                        Ws_t[:, 260:264],
                        start=True,
                        stop=True,
                    )
                    nc.vector.tensor_copy(s2st[:, t2 * 4 : (t2 + 1) * 4], ps2[:])

            # ---- phase 2: per-window edge processing ----
            with (
                tc.tile_pool(name="pG", bufs=2) as pG,
                tc.tile_pool(name="pI", bufs=2) as pI,
                tc.tile_pool(name="pIT", bufs=2) as pIT,
                tc.tile_pool(name="pD", bufs=2) as pD,
                tc.tile_pool(name="pe", bufs=2) as pe,
                tc.tile_pool(name="pps", bufs=2, space="PSUM") as pps,
                tc.tile_pool(name="ppsT", bufs=1, space="PSUM") as ppsT,
                tc.tile_pool(name="ppsE", bufs=2, space="PSUM") as ppsE,
                tc.tile_pool(name="pf", bufs=2) as pf,
            ):
                for w in range(NWIN):
                    WB = woff[w + 1] - woff[w]
                    j0 = woff[w]

                    # indicator Ind[slot, rank] and its transpose, via IS_EQ
                    Ind = pI.tile([128, WSUBMAX * 128], BF16, tag="Ind")
                    Ind3 = Ind[:, 0 : WB * 128].rearrange(
                        "p (s c) -> p s c", c=128
                    )
                    rb, ib = broadcast_tensor_aps(
                        rr_all[:, j0 : j0 + WB].rearrange(
                            "p (s o) -> p s o", o=1
                        ),
                        iota_t[:].rearrange("(p o) c -> p o c", o=1),
                    )
                    nc.vector.tensor_tensor(Ind3, rb, ib, EQ)

                    psT = ppsT.tile([128, WSUBMAX * 128], BF16, tag="psT")
                    psT3 = psT[:, 0 : WB * 128].rearrange(
                        "p (s c) -> p s c", c=128
                    )
                    for j in range(WB):
                        nc.tensor.transpose(
                            psT3[:, j, :], Ind3[:, j, :], ident_t[:]
                        )
                    IndT = pIT.tile([128, WSUBMAX * 128], BF16, tag="IndT")
                    nc.vector.tensor_copy(IndT[:, 0 : WB * 128], psT[:, 0 : WB * 128])
                    IndT3 = IndT[:, 0 : WB * 128].rearrange(
                        "p (s c) -> p s c", c=128
                    )

                    # s_dst expansion: Dex[slot, k] = s_dst[rank(slot), k]
                    psE = ppsE.tile([128, WSUBMAX * 4], F32, tag="psE")
                    for j in range(WB):
                        nc.tensor.matmul(
                            psE[:, 4 * j : 4 * j + 4],
                            IndT3[:, j, :],
                            s2st[:, 4 * w : 4 * w + 4],
                            start=True,
                            stop=True,
                        )
                    Dex = pD.tile([128, WSUBMAX * 4], BF16, tag="Dex")
                    nc.vector.tensor_copy(Dex[:, 0 : WB * 4], psE[:, 0 : WB * 4])
                    D3 = Dex[:, 0 : WB * 4].rearrange("p (s f) -> p s f", f=4)

                    # gather per-edge rows (pad slots have idx=-1: skipped)
                    G = pG.tile([128, WSUBMAX * BROW], BF16, tag="G")
                    if w < 2:
                        nc.vector.memset(G[:], 0.0)
                    G3 = G[:, 0 : WB * BROW].rearrange(
                        "p (s e) -> p s e", e=BROW
                    )
                    for b in range(NBLK):
                        ws = WS[w][b]
                        if ws == 0:
                            continue
                        gj0 = j0 + boff[w][b]
                        nidx = 128 * ws
                        lo = SBLK * b
                        hi = min(SBLK * (b + 1), NPAD)
                        nc.gpsimd.dma_gather(
                            G3[:, boff[w][b] : boff[w][b] + ws, :],
                            tabA[lo:hi, :],
                            gi_all[:, gj0 * 8 : (gj0 + ws) * 8],
                            nidx,
                            nidx,
                            BROW,
                        )

                    # e = clip(exp(lrelu(s_src + s_dst)))
                    L = pe.tile([128, WSUBMAX * 4], F32, tag="L")
                    L3 = L[:, 0 : WB * 4].rearrange("p (s f) -> p s f", f=4)
                    nc.vector.tensor_tensor(
                        L3, G3[:, :, 256:260], D3[:, :, 0:4], ADD
                    )
                    L2 = pe.tile([128, WSUBMAX * 4], F32, tag="L2")
                    nc.vector.scalar_tensor_tensor(
                        L2[:, 0 : WB * 4],
                        L[:, 0 : WB * 4],
                        SLOPE,
                        L[:, 0 : WB * 4],
                        MUL,
                        MAX,
                    )
                    Et = pe.tile([128, WSUBMAX * 4], F32, tag="Et")
                    nc.scalar.activation(
                        Et[:, 0 : WB * 4],
                        L2[:, 0 : WB * 4],
                        mybir.ActivationFunctionType.Exp,
                    )
                    Ec = pe.tile([128, WSUBMAX * 4], F32, tag="Ec")
                    nc.vector.tensor_scalar(
                        Ec[:, 0 : WB * 4], Et[:, 0 : WB * 4], CLIP_HI, None, MIN
                    )
                    Eb = pe.tile([128, WSUBMAX * 4], BF16, tag="Eb")
                    nc.vector.tensor_scalar(
                        Eb[:, 0 : WB * 4], Ec[:, 0 : WB * 4], CLIP_LO, None, MAX
                    )
                    E3 = Eb[:, 0 : WB * 4].rearrange("p (s f) -> p s f", f=4)

                    # weight messages in place; append e to columns 256:260
                    for k in range(HEADS):
                        gk = G3[:, :, OUT_DIM * k : OUT_DIM * (k + 1)]
                        ek, gkb = broadcast_tensor_aps(E3[:, :, k : k + 1], gk)
                        nc.vector.tensor_tensor(gkb, gkb, ek, MUL)
                    nc.vector.tensor_copy(G3[:, :, 256:260], E3)

                    # segment matmuls
                    ps = pps.tile([128, USED], F32, tag="mps")
                    for j in range(WB):
                        nc.tensor.matmul(
                            ps[:],
                            Ind3[:, j, :],
                            G3[:, j, 0:USED],
                            start=(j == 0),
                            stop=(j == WB - 1),
                        )

                    # flush + normalize: out = 0.25 * sum_k num_k / den_k
                    F = pf.tile([128, USED], F32, tag="F")
                    nc.vector.tensor_copy(F[:], ps[:])
                    R = pf.tile([128, 4], F32, tag="R")
                    nc.vector.tensor_scalar(R[:], F[:, 256:260], 1e-30, None, MAX)
                    R2 = pf.tile([128, 4], F32, tag="R2")
                    nc.vector.reciprocal(R2[:], R[:])
                    nc.vector.tensor_scalar(R2[:], R2[:], 1.0 / HEADS, None, MUL)
                    O = pf.tile([128, OUT_DIM], F32, tag="O")
                    nc.vector.tensor_scalar(
                        O[:], F[:, 0:OUT_DIM], R2[:, 0:1], None, MUL
                    )
                    for k in range(1, HEADS):
                        nc.vector.scalar_tensor_tensor(
                            O[:],
                            F[:, OUT_DIM * k : OUT_DIM * (k + 1)],
                            R2[:, k : k + 1],
                            O[:],
                            MUL,
                            ADD,
                        )
                    nc.sync.dma_start(out_d[w * 128 : (w + 1) * 128, :], O[:])

    nc.compile()
    _split_waits(nc)
    return nc


def kernel(h, edges, W, a):
    shared, per_core, WS = _preprocess(h, edges, W, a)
    nc = _build(WS)
    in_maps = [{**shared, **pc} for pc in per_core]
    r = run_bass_kernel_spmd(
        nc, in_maps, list(range(NC)), trace=bool(os.environ.get("GAT_TRACE"))
    )
    res = r.results
    global _last_results, _last_exec_ns, _last_bkr
    _last_results = res
    _last_exec_ns = r.exec_time_ns
    _last_bkr = r
    out = np.zeros((N, OUT_DIM), np.float32)
    for m in range(NC):
        out[m * NPC : (m + 1) * NPC] = res[m]["out"][:NPC]
    return out
